# revision 14
# baseline (speedup 1.0000x reference)
"""Concordance CC (segment_reduce) Trainium2 Bass kernel.

Problem: y_true, y_pred [256, 65536] f32, prefix-validity mask [256, 65536] i32.
Per row: masked means/variances/covariance (ddof=1), ccc = 2*cov /
(var_t + var_p + 2*(mean_t - mean_p)); output = mean(ccc) (scalar f32).

Strategy (data parallel over B, 8 cores x 32 rows):
Every per-row statistic is an inner product over T of columns from
W = [a, b, m] with a = y_true*mask, b = y_pred*mask, m = mask:
  S2t=a.a  Stp=a.b  S1t=a.m  S2p=b.b  S1p=b.m  L=m.m
Each core computes Gram matrices W^T W on the TensorEngine (PSUM-accumulated
over T), with rows processed in two groups of 16 (48x48 Grams) so staging
stays within SBUF at a 32768-wide T-block (1KB DMA descriptor runs).
All loads ride the SWDGE (gpsimd) DMA path, which packs descriptors into
4KB packets (~2.5-3x the HWDGE descriptor efficiency on this pattern).
The VectorEngine builds the masked bf16 operands, ScalarE converts the
mask; host does the O(B) scalar epilogue.
"""

import numpy as np

import concourse.bass as bass
import concourse.tile as tile
from concourse import mybir
from concourse.bass_utils import run_bass_kernel_spmd

# ---------------------------------------------------------------- constants
B, T = 256, 65536
NCORES = 8
R = B // NCORES            # rows per core = 32
GROUPS = 2                 # row groups per core
R2 = R // GROUPS           # rows per group = 16
TB = 32768                 # T-block size
NBLK = T // TB             # 2
JB = TB // 128             # chunk positions per row per block = 256
GCOLS = 3 * R2             # 48 Gram columns per group: [a_0..15, b_0..15, m_0..15]
GFREE = JB * GCOLS         # 12288 free elems in a G tile

FP = mybir.dt.bfloat16     # Gram operand precision (PE-native, 1 cyc/col)


def split_multi_waits(nc: bass.Bass) -> int:
    """This container's walrus build accepts at most ONE sync-wait per
    instruction, but Tile's sem assignment attaches all required waits to
    the consuming instruction. Hoist the excess onto same-engine NoOps
    inserted immediately before it (sequencers execute in order, so the
    waits are still satisfied before the instruction issues)."""
    n_split = 0
    for f in nc.m.functions:
        for bb in f.blocks:
            insts = bb.instructions
            out = []
            for inst in insts:
                si = inst.sync_info
                if si is not None and si.on_wait and len(si.on_wait) > 1:
                    waits = list(si.on_wait)
                    for w in waits[:-1]:
                        nop = mybir.InstNoOp(
                            name=f"I-wsplit-{nc.next_id()}", ins=[], outs=[]
                        )
                        nop.engine = inst.engine
                        nop.sync_info = mybir.SyncInfo(on_wait=[w], on_update=[])
                        out.append(nop)
                        n_split += 1
                    inst.sync_info = mybir.SyncInfo(
                        on_wait=[waits[-1]], on_update=list(si.on_update or [])
                    )
                out.append(inst)
            bb.instructions = out
    return n_split


def build_nc() -> bass.Bass:
    nc = bass.Bass()
    yt = nc.dram_tensor("y_true", [R, T], mybir.dt.float32, kind="ExternalInput")
    yp = nc.dram_tensor("y_pred", [R, T], mybir.dt.float32, kind="ExternalInput")
    mk = nc.dram_tensor("mask", [R, T], mybir.dt.int32, kind="ExternalInput")
    gram = nc.dram_tensor("gram", [GCOLS, GROUPS * GCOLS], mybir.dt.float32,
                          kind="ExternalOutput")

    with tile.TileContext(nc) as tc:
        with (
            tc.tile_pool(name="gpool", bufs=2) as gpool,
            tc.tile_pool(name="stage", bufs=2) as stage,
            # tp is consumed last (TT-b), so its slot release gates the DMA
            # ring two units ahead; give it one extra slot.
            tc.tile_pool(name="stagep", bufs=3) as stagep,
            tc.tile_pool(name="psum", bufs=1, space="PSUM") as psum,
            tc.tile_pool(name="outp", bufs=1) as outp,
        ):
            ps0 = psum.tile([GCOLS, GCOLS], mybir.dt.float32)
            ps1 = psum.tile([GCOLS, GCOLS], mybir.dt.float32)
            ps = [ps0, ps1]
            for tb in range(NBLK):
                for grp in range(GROUPS):
                    # G is chunk-major: G[p, ci*GCOLS + k] so each matmul
                    # chunk's operand G[:, ci*48:(ci+1)*48] is contiguous
                    # (strided PE APs measured ~8x slower). The DVE writes
                    # are strided instead (1x REGULAR mode either way).
                    g = gpool.tile([128, GFREE], FP)
                    tt = stage.tile([128, R2 * JB], mybir.dt.float32)
                    tp = stagep.tile([128, R2 * JB], mybir.dt.float32)
                    tm = stage.tile([128, R2 * JB], mybir.dt.int32)

                    lo, hi = tb * TB, (tb + 1) * TB
                    r0 = grp * R2
                    # staging layout: tile[p, r*JB + c] = src[r0+r, lo + p*JB + c]
                    src = lambda h: h[r0 : r0 + R2, lo:hi].rearrange(
                        "r (p c) -> p r c", p=128
                    )
                    dst = lambda t_: t_[:, :].rearrange("p (r c) -> p r c", r=R2)
                    # mask first: the ScalarE m-copy and both TTs depend on it
                    nc.gpsimd.dma_start(out=dst(tm), in_=src(mk))
                    nc.gpsimd.dma_start(out=dst(tt), in_=src(yt))
                    nc.gpsimd.dma_start(out=dst(tp), in_=src(yp))

                    # [p][c][r] iteration: G-side inner runs are the 16
                    # contiguous cols of one tensor within a chunk (strided
                    # inner writes measured 4x slower; strided reads are ok).
                    gv = g[:, :].rearrange("p (c k) -> p c k", k=GCOLS)
                    ga = gv[:, :, 0:R2]
                    gb = gv[:, :, R2 : 2 * R2]
                    gm = gv[:, :, 2 * R2 : 3 * R2]
                    stg = lambda t_: t_[:, :].rearrange("p (r c) -> p c r", r=R2)
                    # m (i32 -> bf16) on the otherwise-idle ScalarE, then
                    # a = y_true*m, b = y_pred*m on VectorE
                    nc.scalar.copy(out=gm, in_=stg(tm))
                    nc.vector.tensor_mul(out=ga, in0=stg(tt), in1=gm)
                    nc.vector.tensor_mul(out=gb, in0=stg(tp), in1=gm)

                    for ci in range(JB):
                        w = g[:, ci * GCOLS : (ci + 1) * GCOLS]
                        nc.tensor.matmul(
                            ps[grp][:, :],
                            lhsT=w,
                            rhs=w,
                            start=(tb == 0 and ci == 0),
                            stop=(tb == NBLK - 1 and ci == JB - 1),
                        )

            out_t = outp.tile([GCOLS, GROUPS * GCOLS], mybir.dt.float32)
            for grp in range(GROUPS):
                nc.vector.tensor_copy(
                    out=out_t[:, grp * GCOLS : (grp + 1) * GCOLS], in_=ps[grp][:, :]
                )
            nc.sync.dma_start(out=gram[:, :], in_=out_t[:, :])
    split_multi_waits(nc)
    return nc


_NC_CACHE = None


def _get_nc():
    global _NC_CACHE
    if _NC_CACHE is None:
        _NC_CACHE = build_nc()
    return _NC_CACHE


def _ccc_from_grams(grams: list[np.ndarray]) -> np.ndarray:
    idx = np.arange(R2)
    total = 0.0
    for gm_ in grams:
        for grp in range(GROUPS):
            g = gm_[:, grp * GCOLS : (grp + 1) * GCOLS].astype(np.float64)
            s2t = g[idx, idx]
            stp = g[idx, R2 + idx]
            s1t = g[idx, 2 * R2 + idx]
            s2p = g[R2 + idx, R2 + idx]
            s1p = g[R2 + idx, 2 * R2 + idx]
            ell = g[2 * R2 + idx, 2 * R2 + idx]
            mean_t = s1t / ell
            mean_p = s1p / ell
            denom = ell - 1.0
            var_t = (s2t - s1t * s1t / ell) / denom
            var_p = (s2p - s1p * s1p / ell) / denom
            cov = (stp - s1t * s1p / ell) / denom
            ccc = 2.0 * cov / (var_t + var_p + (mean_t - mean_p) * 2.0)
            total += ccc.sum()
    return np.float32(total / B)


def kernel(y_true, y_pred, mask) -> np.ndarray:
    y_true = np.ascontiguousarray(np.asarray(y_true, dtype=np.float32))
    y_pred = np.ascontiguousarray(np.asarray(y_pred, dtype=np.float32))
    mask = np.ascontiguousarray(np.asarray(mask, dtype=np.int32))

    nc = _get_nc()
    in_maps = [
        {
            "y_true": y_true[c * R : (c + 1) * R],
            "y_pred": y_pred[c * R : (c + 1) * R],
            "mask": mask[c * R : (c + 1) * R],
        }
        for c in range(NCORES)
    ]
    res = run_bass_kernel_spmd(nc, in_maps, core_ids=list(range(NCORES)))
    grams = [res.results[c]["gram"] for c in range(NCORES)]
    return _ccc_from_grams(grams)
